# revision 1
# baseline (speedup 1.0000x reference)
"""Multi-head attention (B=4, S=2048, D=2048, H=16, dk=128) on 8 TRN2 NeuronCores.

Sharding: core c handles batch b = c // 2 and query-half q = c % 2 (1024 query
rows).  No collectives: the K/V projections for the full sequence are computed
on both cores of a batch pair (33% extra matmul FLOPs on K/V proj only).

Layout strategy (zero on-chip transposes):
  - host supplies x[b]^T ("xt", [D, S]) with the core's query columns rolled to
    the front (softmax over keys is permutation invariant, so the rolled key
    order is harmless as long as Q/K/V all come from the same xt).
  - Q^T, K^T computed transposed:  lhsT = w[:, head] col-block, rhs = xt.
  - V computed natural:           lhsT = xt col-slice,        rhs = w_v chunk.
  - scores computed transposed:   lhsT = K^T_h slice, rhs = Q^T_h  -> [sk, sq].
  - P^T = exp(scores^T * 1/sqrt(dk)) with NO max subtraction (logits are
    ~N(0,1) here; exp stays in a safe fp32 range).
  - row sums via ones-matmul (partition reduction on the PE).
  - attn_out^T = V_h^T @ P^T  (lhsT = V_h natural tile, rhs = P^T).
  - out = (attn_out @ w_o) with the 1/rowsum folded in by scaling attn_out^T
    columns via a PE-broadcast reciprocal (1/sum commutes with the w_o matmul).

Matmuls run in float32r (full PE rate at N>=256; fp32 layout with the mantissa
rounded to 11 bits).  The BIR verifier requires every matmul operand to be
*produced* as float32r, so operand tensors are float32r end-to-end and the
host pre-rounds the external inputs.  PSUM accumulation stays full fp32.
"""

import os
import sys

import numpy as np

for _p in ("/opt/trn_rl_repo", "/root/.axon_site/_ro/trn_rl_repo"):
    if os.path.isdir(_p) and _p not in sys.path:
        sys.path.insert(0, _p)

P = 128

_CACHE = {}


def _round_fp32r(a):
    """Round fp32 array to the fp32r grid (11-bit mantissa, RNE)."""
    u = np.ascontiguousarray(a, dtype=np.float32).view(np.uint32)
    r = (u + 0x7FF + ((u >> 12) & 1)) & np.uint32(0xFFFFF000)
    return r.view(np.float32)


def build_nc(D=2048, S=2048, SQ=1024, use_f32r=True):
    """Build the single-core Bass program (SPMD: identical on all cores)."""
    from contextlib import ExitStack

    import concourse.tile as tile
    from concourse import bacc, mybir

    F32 = mybir.dt.float32
    OD = mybir.dt.float32r if use_f32r else mybir.dt.float32  # matmul operand dtype
    Exp = mybir.ActivationFunctionType.Exp

    H = D // P          # heads == d-tiles (dk == P == 128)
    ST = S // P         # key tiles
    SQT = SQ // P       # query tiles
    NSKC = S // 512     # K^T projection free-dim chunks
    NSQC = max(1, SQ // 512)
    SQCW = SQ // NSQC   # query chunk width (<= 512)
    VC = 256            # w_v streaming chunk width
    NVC = D // VC
    OC = 512            # w_o streaming chunk width
    NOC = D // OC
    scale = float(1.0 / np.sqrt(128.0))

    nc = bacc.Bacc("TRN2", target_bir_lowering=False, debug=False)

    xt_d = nc.dram_tensor("xt", [D, S], OD, kind="ExternalInput").ap()
    ones_d = nc.dram_tensor("ones", [P, P], OD, kind="ExternalInput").ap()
    wq_d = nc.dram_tensor("wq", [H, D, P], OD, kind="ExternalInput").ap()
    wk_d = nc.dram_tensor("wk", [H, D, P], OD, kind="ExternalInput").ap()
    wv_d = nc.dram_tensor("wv", [NVC, D, VC], OD, kind="ExternalInput").ap()
    wo_d = nc.dram_tensor("wo", [NOC, D, OC], OD, kind="ExternalInput").ap()
    out_d = nc.dram_tensor("out", [SQ, D], F32, kind="ExternalOutput").ap()

    mm = nc.tensor.matmul

    with tile.TileContext(nc) as tc, \
            nc.allow_low_precision(reason="fp32r matmul operands (11-bit mantissa)"):
        with ExitStack() as octx:
            dram = octx.enter_context(tc.tile_pool(name="scratch", bufs=1, space="DRAM"))
            vs = [dram.tile([P, D], OD, name=f"vs{t}") for t in range(ST)]
            atds = [dram.tile([P, SQ], OD, name=f"atds{h}") for h in range(H)]

            const = octx.enter_context(tc.tile_pool(name="const", bufs=1))
            ones_sb = const.tile([P, P], OD)
            nc.sync.dma_start(out=ones_sb[:], in_=ones_d[:])

            mainctx = octx.enter_context(ExitStack())
            xt_pool = mainctx.enter_context(tc.tile_pool(name="xtp", bufs=H))

            # ---------------- Phase V: V = x @ w_v (natural layout) ----------------
            with ExitStack() as ctx:
                wvp = ctx.enter_context(tc.tile_pool(name="wvp", bufs=1))
                ev1 = ctx.enter_context(tc.tile_pool(name="ev1", bufs=1))
                psV = ctx.enter_context(tc.tile_pool(name="psV", bufs=1, space="PSUM"))

                xt_sb = []
                wvb0 = None
                for dt in range(H):
                    xts = xt_pool.tile([P, S], OD, name=f"xts{dt}", tag="xt")
                    nc.sync.dma_start(out=xts[:], in_=xt_d[dt * P:(dt + 1) * P, :])
                    xt_sb.append(xts)
                    if dt == 0:
                        # first w_v chunk rides right behind xt tile 0 in the DMA
                        # queue: the dt=0 slice unblocks the very first matmul
                        wvb0 = wvp.tile([P, H, VC], OD, name="wvb0", tag="wv", bufs=2)
                        ap0 = wv_d[0].rearrange("(t p) n -> p t n", p=P)
                        nc.sync.dma_start(out=wvb0[:, 0:1, :], in_=ap0[:, 0:1, :])
                        nc.sync.dma_start(out=wvb0[:, 1:, :], in_=ap0[:, 1:, :])

                for nvc in range(NVC):
                    if nvc == 0:
                        wvb = wvb0
                    else:
                        wvb = wvp.tile([P, H, VC], OD, name="wvb", tag="wv", bufs=2)
                        nc.sync.dma_start(
                            out=wvb[:], in_=wv_d[nvc].rearrange("(t p) n -> p t n", p=P))
                    for svt in range(ST):
                        psv = psV.tile([P, VC], F32, name="psv", tag="psv", bufs=4)
                        for dt in range(H):
                            mm(psv[:], xt_sb[dt][:, svt * P:(svt + 1) * P], wvb[:, dt, :],
                               start=(dt == 0), stop=(dt == H - 1))
                        vev = ev1.tile([P, VC], OD, name="vev", tag="evv", bufs=4)
                        nc.vector.tensor_copy(vev[:], psv[:])
                        nc.sync.dma_start(
                            out=vs[svt][:, nvc * VC:(nvc + 1) * VC], in_=vev[:])

            # ------ Fused: per-head K/Q projection (SBUF-resident) + attention ------
            with ExitStack() as ctx:
                wqk = ctx.enter_context(tc.tile_pool(name="wqk", bufs=1))
                iok = ctx.enter_context(tc.tile_pool(name="iok", bufs=1))
                pt_pool = ctx.enter_context(tc.tile_pool(name="ptp", bufs=1))
                sm2 = ctx.enter_context(tc.tile_pool(name="sm2", bufs=1))
                ps_kq = ctx.enter_context(tc.tile_pool(name="pskq", bufs=2, space="PSUM"))
                ps_pt = ctx.enter_context(tc.tile_pool(name="pspt", bufs=3, space="PSUM"))
                ps_ov = ctx.enter_context(tc.tile_pool(name="psov", bufs=2, space="PSUM"))
                ps_sm = ctx.enter_context(tc.tile_pool(name="pssm", bufs=1, space="PSUM"))

                k2s, q2s, v2s = {}, {}, {}

                def emit_kq_proj(h):
                    wkb = wqk.tile([P, H, P], OD, name="wkb", tag="w", bufs=2)
                    nc.sync.dma_start(
                        out=wkb[:], in_=wk_d[h].rearrange("(t p) n -> p t n", p=P))
                    k2 = iok.tile([P, S], OD, name="k2", tag="k", bufs=2)
                    for g in range(NSKC // 2):
                        psk = [ps_kq.tile([P, 512], F32, name=f"psk{c}", tag="ps", bufs=2)
                               for c in range(2)]
                        for dt in range(H):
                            for c in range(2):
                                sk = (2 * g + c) * 512
                                mm(psk[c][:], wkb[:, dt, :], xt_sb[dt][:, sk:sk + 512],
                                   start=(dt == 0), stop=(dt == H - 1))
                        for c in range(2):
                            sk = (2 * g + c) * 512
                            nc.vector.tensor_copy(k2[:, sk:sk + 512], psk[c][:])
                    wqb = wqk.tile([P, H, P], OD, name="wqb", tag="w", bufs=2)
                    nc.sync.dma_start(
                        out=wqb[:], in_=wq_d[h].rearrange("(t p) n -> p t n", p=P))
                    q2 = iok.tile([P, SQ], OD, name="q2", tag="q", bufs=2)
                    psq = [ps_kq.tile([P, SQCW], F32, name=f"psq{c}", tag="ps", bufs=2)
                           for c in range(NSQC)]
                    for dt in range(H):
                        for c in range(NSQC):
                            mm(psq[c][:], wqb[:, dt, :],
                               xt_sb[dt][:, c * SQCW:(c + 1) * SQCW],
                               start=(dt == 0), stop=(dt == H - 1))
                    for c in range(NSQC):
                        nc.vector.tensor_copy(
                            q2[:, c * SQCW:(c + 1) * SQCW], psq[c][:])
                    v2 = iok.tile([P, ST, P], OD, name="v2", tag="v", bufs=2)
                    for t in range(ST):
                        nc.sync.dma_start(
                            out=v2[:, t, :], in_=vs[t][:, h * P:(h + 1) * P])
                    k2s[h], q2s[h], v2s[h] = k2, q2, v2

                LEAD = 2
                def emit_attention(h):
                    k2, q2, v2 = k2s[h], q2s[h], v2s[h]
                    for sqc in range(NSQC):
                        pso = ps_ov.tile([P, SQCW], F32, name="pso")
                        psb = ps_sm.tile([P, SQCW], F32, name="psb")
                        ptts = [None] * ST
                        for t in range(ST + LEAD):
                            if t < ST:
                                pst = ps_pt.tile([P, SQCW], F32, name="pst")
                                mm(pst[:], k2[:, t * P:(t + 1) * P],
                                   q2[:, sqc * SQCW:(sqc + 1) * SQCW],
                                   start=True, stop=True)
                                ptt = pt_pool.tile([P, SQCW], OD, name="ptt",
                                                   tag="pt", bufs=5)
                                nc.scalar.activation(ptt[:], pst[:], Exp, scale=scale)
                                ptts[t] = ptt
                            if t >= LEAD:
                                u = t - LEAD
                                mm(psb[:], ones_sb[:], ptts[u][:],
                                   start=(u == 0), stop=(u == ST - 1))
                                mm(pso[:], v2[:, u, :], ptts[u][:],
                                   start=(u == 0), stop=(u == ST - 1))
                        rbc = sm2.tile([P, SQCW], F32, name="rbc", tag="rbc", bufs=2)
                        nc.vector.reciprocal(rbc[:], psb[:])
                        atv = sm2.tile([P, SQCW], OD, name="atv", tag="atv", bufs=2)
                        nc.vector.tensor_mul(atv[:], pso[:], rbc[:])
                        nc.sync.dma_start(
                            out=atds[h][:, sqc * SQCW:(sqc + 1) * SQCW], in_=atv[:])

                emit_kq_proj(0)
                for h in range(H):
                    if h + 1 < H:
                        emit_kq_proj(h + 1)
                    emit_attention(h)

            # close xt (and fused-phase) pools before the out-projection scope
            mainctx.close()

            # ---------------- Out-projection ----------------
            with ExitStack() as ctx:
                at3p = ctx.enter_context(tc.tile_pool(name="at3p", bufs=H))
                wo3 = ctx.enter_context(tc.tile_pool(name="wo3", bufs=1))
                ev3 = ctx.enter_context(tc.tile_pool(name="ev3", bufs=1))
                ps3p = ctx.enter_context(tc.tile_pool(name="ps3p", bufs=4, space="PSUM"))

                at3 = []
                for h in range(H):
                    a3 = at3p.tile([P, SQ], OD, name=f"a3{h}", tag="a3")
                    nc.sync.dma_start(out=a3[:], in_=atds[h][:])
                    at3.append(a3)

                for oc in range(NOC):
                    wob = wo3.tile([P, H, OC], OD, name="wob", tag="wo", bufs=2)
                    nc.sync.dma_start(
                        out=wob[:], in_=wo_d[oc].rearrange("(t p) n -> p t n", p=P))
                    for sqt in range(SQT):
                        ps3 = ps3p.tile([P, OC], F32, name="ps3")
                        for h in range(H):
                            mm(ps3[:], at3[h][:, sqt * P:(sqt + 1) * P],
                               wob[:, h, :], start=(h == 0), stop=(h == H - 1))
                        oev = ev3.tile([P, OC], F32, name="oev", tag="oev", bufs=6)
                        nc.vector.tensor_copy(oev[:], ps3[:])
                        nc.sync.dma_start(
                            out=out_d[sqt * P:(sqt + 1) * P, oc * OC:(oc + 1) * OC],
                            in_=oev[:])

    nc.compile()
    return nc


def prep_inputs(x, w_q, w_k, w_v, w_o, D=2048, S=2048, SQ=1024, n_cores=8,
                use_f32r=True):
    """Host-side shard + re-layout. Returns in_maps for run_bass_kernel_spmd."""
    H = D // P
    NVC = D // 256
    NOC = D // 512
    rnd = _round_fp32r if use_f32r else (lambda a: np.ascontiguousarray(a, np.float32))
    wq_cb = rnd(w_q.reshape(D, H, P).transpose(1, 0, 2))
    wk_cb = rnd(w_k.reshape(D, H, P).transpose(1, 0, 2))
    wv_cb = rnd(w_v.reshape(D, NVC, 256).transpose(1, 0, 2))
    wo_cb = rnd(w_o.reshape(D, NOC, 512).transpose(1, 0, 2))
    in_maps = []
    for c in range(n_cores):
        b, half = divmod(c, 2)
        xt = x[b].T  # [D, S]
        # roll this core's query columns to the front
        xt = rnd(np.roll(xt, -half * SQ, axis=1))
        in_maps.append({
            "xt": xt, "wq": wq_cb, "wk": wk_cb, "wv": wv_cb, "wo": wo_cb,
            "ones": np.ones((P, P), dtype=np.float32),
        })
    return in_maps


def run(x, w_q, w_k, w_v, w_o, trace=False, use_f32r=True):
    from concourse.bass_utils import run_bass_kernel_spmd

    B, S, D = x.shape
    n_cores = 8
    SQ = (B * S) // n_cores
    key = (D, S, SQ, use_f32r)
    if key not in _CACHE:
        _CACHE[key] = build_nc(D=D, S=S, SQ=SQ, use_f32r=use_f32r)
    nc = _CACHE[key]
    in_maps = prep_inputs(x, w_q, w_k, w_v, w_o, D=D, S=S, SQ=SQ,
                          n_cores=n_cores, use_f32r=use_f32r)
    res = run_bass_kernel_spmd(nc, in_maps, core_ids=list(range(n_cores)), trace=trace)
    out = np.empty((B, S, D), dtype=np.float32)
    for c in range(n_cores):
        b, half = divmod(c, 2)
        out[b, half * SQ:(half + 1) * SQ, :] = res.results[c]["out"]
    return out, res


def kernel(x, w_q, w_k, w_v, w_o):
    out, _ = run(np.asarray(x), np.asarray(w_q), np.asarray(w_k),
                 np.asarray(w_v), np.asarray(w_o))
    return out



# revision 2
# speedup vs baseline: 1.2806x; 1.2806x over previous
"""Multi-head attention (B=4, S=2048, D=2048, H=16, dk=128) on 8 TRN2 NeuronCores.

Sharding: core c handles batch b = c // 2 and query-half q = c % 2 (1024 query
rows).  No collectives: the K/V projections for the full sequence are computed
on both cores of a batch pair (33% extra matmul FLOPs on K/V proj only).

v2 changes vs v1:
  - all matmul operands in bfloat16 (PSUM accumulation stays fp32).  fp32r
    matmuls are weight-load bound: the PE's self-loading LDWEIGHTS takes
    ~224ns per 128x128 fp32r tile while an N=512 matmul computes in 213ns.
    bf16 halves the weight-load time (~112ns), which hides fully under the
    compute of N=512 matmuls -> PE runs at ~1 col/cycle.
  - every matmul has a 512-wide moving dim (V projection now uses 512-col
    weight chunks instead of 256).
  - V and the per-head attention outputs stay SBUF-resident (bf16 halves the
    footprint; no DRAM bounce round-trips).

Layout strategy (zero on-chip transposes):
  - host supplies x[b]^T ("xt", [D, S]) with the core's query columns rolled to
    the front (softmax over keys is permutation invariant, so the rolled key
    order is harmless as long as Q/K/V all come from the same xt).
  - Q^T, K^T computed transposed:  lhsT = w[:, head] col-block, rhs = xt.
  - V computed natural:           lhsT = xt col-slice,        rhs = w_v chunk.
  - scores computed transposed:   lhsT = K^T_h slice, rhs = Q^T_h  -> [sk, sq].
  - P^T = exp(scores^T * 1/sqrt(dk)) with NO max subtraction (logits are
    ~N(0,1) here; exp stays in a safe fp32 range).
  - row sums via ones-matmul (partition reduction on the PE).
  - attn_out^T = V_h^T @ P^T  (lhsT = V_h natural tile, rhs = P^T).
  - out = (attn_out @ w_o) with the 1/rowsum folded in by scaling attn_out^T
    columns via a PE-broadcast reciprocal (1/sum commutes with the w_o matmul).
"""

import os
import sys

import numpy as np

for _p in ("/opt/trn_rl_repo", "/root/.axon_site/_ro/trn_rl_repo"):
    if os.path.isdir(_p) and _p not in sys.path:
        sys.path.insert(0, _p)

P = 128

_CACHE = {}


def _bf16(a):
    import ml_dtypes

    return np.ascontiguousarray(a, dtype=np.float32).astype(ml_dtypes.bfloat16)


def build_nc(D=2048, S=2048, SQ=1024):
    """Build the single-core Bass program (SPMD: identical on all cores)."""
    from contextlib import ExitStack

    import concourse.tile as tile
    from concourse import bacc, mybir

    F32 = mybir.dt.float32
    BF16 = mybir.dt.bfloat16
    Exp = mybir.ActivationFunctionType.Exp

    H = D // P          # heads == d-tiles (dk == P == 128)
    ST = S // P         # key tiles
    SQT = SQ // P       # query tiles
    NSKC = S // 512     # K^T projection free-dim chunks
    NSQC = max(1, SQ // 512)
    SQCW = SQ // NSQC   # query chunk width (<= 512)
    VC = 512            # w_v streaming chunk width
    NVC = D // VC
    OC = 512            # w_o streaming chunk width
    NOC = D // OC
    scale = float(1.0 / np.sqrt(128.0))

    nc = bacc.Bacc("TRN2", target_bir_lowering=False, debug=False)

    xt_d = nc.dram_tensor("xt", [D, S], BF16, kind="ExternalInput").ap()
    ones_d = nc.dram_tensor("ones", [P, P], BF16, kind="ExternalInput").ap()
    wq_d = nc.dram_tensor("wq", [H, D, P], BF16, kind="ExternalInput").ap()
    wk_d = nc.dram_tensor("wk", [H, D, P], BF16, kind="ExternalInput").ap()
    wv_d = nc.dram_tensor("wv", [NVC, D, VC], BF16, kind="ExternalInput").ap()
    wo_d = nc.dram_tensor("wo", [NOC, D, OC], BF16, kind="ExternalInput").ap()
    out_d = nc.dram_tensor("out", [SQ, D], F32, kind="ExternalOutput").ap()

    mm = nc.tensor.matmul

    with tile.TileContext(nc) as tc, \
            nc.allow_low_precision(reason="bf16 matmul operands"):
        with ExitStack() as octx:
            const = octx.enter_context(tc.tile_pool(name="const", bufs=1))
            ones_sb = const.tile([P, P], BF16)
            nc.sync.dma_start(out=ones_sb[:], in_=ones_d[:])

            # persistent SBUF residents: x^T tiles, V natural tiles, attn-out^T
            xt_pool = octx.enter_context(tc.tile_pool(name="xtp", bufs=H))
            v_pool = octx.enter_context(tc.tile_pool(name="vp", bufs=ST))
            ao_pool = octx.enter_context(tc.tile_pool(name="aop", bufs=H))

            xt_sb = []
            for dt in range(H):
                xts = xt_pool.tile([P, S], BF16, name=f"xts{dt}", tag="xt")
                nc.sync.dma_start(out=xts[:], in_=xt_d[dt * P:(dt + 1) * P, :])
                xt_sb.append(xts)
            v_sb = [v_pool.tile([P, D], BF16, name=f"vs{t}", tag="v")
                    for t in range(ST)]
            ao_sb = [ao_pool.tile([P, SQ], BF16, name=f"ao{h}", tag="ao")
                     for h in range(H)]

            # ---------------- Phase V: V = x @ w_v (natural layout) ----------------
            with ExitStack() as ctx:
                wvp = ctx.enter_context(tc.tile_pool(name="wvp", bufs=1))
                ev1 = ctx.enter_context(tc.tile_pool(name="ev1", bufs=1))
                psV = ctx.enter_context(tc.tile_pool(name="psV", bufs=1, space="PSUM"))

                for nvc in range(NVC):
                    wvb = wvp.tile([P, H, VC], BF16, name="wvb", tag="wv", bufs=2)
                    nc.sync.dma_start(
                        out=wvb[:], in_=wv_d[nvc].rearrange("(t p) n -> p t n", p=P))
                    for svt in range(ST):
                        psv = psV.tile([P, VC], F32, name="psv", tag="psv", bufs=4)
                        for dt in range(H):
                            mm(psv[:], xt_sb[dt][:, svt * P:(svt + 1) * P], wvb[:, dt, :],
                               start=(dt == 0), stop=(dt == H - 1))
                        nc.vector.tensor_copy(
                            v_sb[svt][:, nvc * VC:(nvc + 1) * VC], psv[:])

            # ------ Fused: per-head K/Q projection (SBUF-resident) + attention ------
            with ExitStack() as ctx:
                wqk = ctx.enter_context(tc.tile_pool(name="wqk", bufs=1))
                iok = ctx.enter_context(tc.tile_pool(name="iok", bufs=1))
                pt_pool = ctx.enter_context(tc.tile_pool(name="ptp", bufs=1))
                sm2 = ctx.enter_context(tc.tile_pool(name="sm2", bufs=1))
                ps_kq = ctx.enter_context(tc.tile_pool(name="pskq", bufs=2, space="PSUM"))
                ps_pt = ctx.enter_context(tc.tile_pool(name="pspt", bufs=3, space="PSUM"))
                ps_ov = ctx.enter_context(tc.tile_pool(name="psov", bufs=2, space="PSUM"))
                ps_sm = ctx.enter_context(tc.tile_pool(name="pssm", bufs=1, space="PSUM"))

                k2s, q2s = {}, {}

                def emit_kq_proj(h):
                    wkb = wqk.tile([P, H, P], BF16, name="wkb", tag="w", bufs=2)
                    nc.sync.dma_start(
                        out=wkb[:], in_=wk_d[h].rearrange("(t p) n -> p t n", p=P))
                    k2 = iok.tile([P, S], BF16, name="k2", tag="k", bufs=2)
                    for g in range(NSKC // 2):
                        psk = [ps_kq.tile([P, 512], F32, name=f"psk{c}", tag="ps", bufs=2)
                               for c in range(2)]
                        for dt in range(H):
                            for c in range(2):
                                sk = (2 * g + c) * 512
                                mm(psk[c][:], wkb[:, dt, :], xt_sb[dt][:, sk:sk + 512],
                                   start=(dt == 0), stop=(dt == H - 1))
                        for c in range(2):
                            sk = (2 * g + c) * 512
                            nc.vector.tensor_copy(k2[:, sk:sk + 512], psk[c][:])
                    wqb = wqk.tile([P, H, P], BF16, name="wqb", tag="w", bufs=2)
                    nc.sync.dma_start(
                        out=wqb[:], in_=wq_d[h].rearrange("(t p) n -> p t n", p=P))
                    q2 = iok.tile([P, SQ], BF16, name="q2", tag="q", bufs=2)
                    psq = [ps_kq.tile([P, SQCW], F32, name=f"psq{c}", tag="ps", bufs=2)
                           for c in range(NSQC)]
                    for dt in range(H):
                        for c in range(NSQC):
                            mm(psq[c][:], wqb[:, dt, :],
                               xt_sb[dt][:, c * SQCW:(c + 1) * SQCW],
                               start=(dt == 0), stop=(dt == H - 1))
                    for c in range(NSQC):
                        nc.vector.tensor_copy(
                            q2[:, c * SQCW:(c + 1) * SQCW], psq[c][:])
                    k2s[h], q2s[h] = k2, q2

                LEAD = 2
                def emit_attention(h):
                    k2, q2 = k2s[h], q2s[h]
                    for sqc in range(NSQC):
                        pso = ps_ov.tile([P, SQCW], F32, name="pso")
                        psb = ps_sm.tile([P, SQCW], F32, name="psb")
                        ptts = [None] * ST
                        for t in range(ST + LEAD):
                            if t < ST:
                                pst = ps_pt.tile([P, SQCW], F32, name="pst")
                                mm(pst[:], k2[:, t * P:(t + 1) * P],
                                   q2[:, sqc * SQCW:(sqc + 1) * SQCW],
                                   start=True, stop=True)
                                ptt = pt_pool.tile([P, SQCW], BF16, name="ptt",
                                                   tag="pt", bufs=5)
                                nc.scalar.activation(ptt[:], pst[:], Exp, scale=scale)
                                ptts[t] = ptt
                            if t >= LEAD:
                                u = t - LEAD
                                mm(psb[:], ones_sb[:], ptts[u][:],
                                   start=(u == 0), stop=(u == ST - 1))
                                mm(pso[:], v_sb[u][:, h * P:(h + 1) * P], ptts[u][:],
                                   start=(u == 0), stop=(u == ST - 1))
                        rbc = sm2.tile([P, SQCW], F32, name="rbc", tag="rbc", bufs=2)
                        nc.vector.reciprocal(rbc[:], psb[:])
                        nc.vector.tensor_mul(
                            ao_sb[h][:, sqc * SQCW:(sqc + 1) * SQCW], pso[:], rbc[:])

                emit_kq_proj(0)
                for h in range(H):
                    if h + 1 < H:
                        emit_kq_proj(h + 1)
                    emit_attention(h)

            # ---------------- Out-projection ----------------
            with ExitStack() as ctx:
                wo3 = ctx.enter_context(tc.tile_pool(name="wo3", bufs=1))
                ev3 = ctx.enter_context(tc.tile_pool(name="ev3", bufs=1))
                ps3p = ctx.enter_context(tc.tile_pool(name="ps3p", bufs=4, space="PSUM"))

                for oc in range(NOC):
                    wob = wo3.tile([P, H, OC], BF16, name="wob", tag="wo", bufs=2)
                    nc.sync.dma_start(
                        out=wob[:], in_=wo_d[oc].rearrange("(t p) n -> p t n", p=P))
                    for sqt in range(SQT):
                        ps3 = ps3p.tile([P, OC], F32, name="ps3")
                        for h in range(H):
                            mm(ps3[:], ao_sb[h][:, sqt * P:(sqt + 1) * P],
                               wob[:, h, :], start=(h == 0), stop=(h == H - 1))
                        oev = ev3.tile([P, OC], F32, name="oev", tag="oev", bufs=6)
                        nc.vector.tensor_copy(oev[:], ps3[:])
                        nc.sync.dma_start(
                            out=out_d[sqt * P:(sqt + 1) * P, oc * OC:(oc + 1) * OC],
                            in_=oev[:])

    nc.compile()
    return nc


def prep_inputs(x, w_q, w_k, w_v, w_o, D=2048, S=2048, SQ=1024, n_cores=8):
    """Host-side shard + re-layout. Returns in_maps for run_bass_kernel_spmd."""
    H = D // P
    NVC = D // 512
    NOC = D // 512
    wq_cb = _bf16(w_q.reshape(D, H, P).transpose(1, 0, 2))
    wk_cb = _bf16(w_k.reshape(D, H, P).transpose(1, 0, 2))
    wv_cb = _bf16(w_v.reshape(D, NVC, 512).transpose(1, 0, 2))
    wo_cb = _bf16(w_o.reshape(D, NOC, 512).transpose(1, 0, 2))
    ones = _bf16(np.ones((P, P), dtype=np.float32))
    in_maps = []
    for c in range(n_cores):
        b, half = divmod(c, 2)
        xt = x[b].T  # [D, S]
        # roll this core's query columns to the front
        xt = _bf16(np.roll(xt, -half * SQ, axis=1))
        in_maps.append({
            "xt": xt, "wq": wq_cb, "wk": wk_cb, "wv": wv_cb, "wo": wo_cb,
            "ones": ones,
        })
    return in_maps


def run(x, w_q, w_k, w_v, w_o, trace=False):
    from concourse.bass_utils import run_bass_kernel_spmd

    B, S, D = x.shape
    n_cores = 8
    SQ = (B * S) // n_cores
    key = (D, S, SQ)
    if key not in _CACHE:
        _CACHE[key] = build_nc(D=D, S=S, SQ=SQ)
    nc = _CACHE[key]
    in_maps = prep_inputs(x, w_q, w_k, w_v, w_o, D=D, S=S, SQ=SQ,
                          n_cores=n_cores)
    res = run_bass_kernel_spmd(nc, in_maps, core_ids=list(range(n_cores)), trace=trace)
    out = np.empty((B, S, D), dtype=np.float32)
    for c in range(n_cores):
        b, half = divmod(c, 2)
        out[b, half * SQ:(half + 1) * SQ, :] = res.results[c]["out"]
    return out, res


def kernel(x, w_q, w_k, w_v, w_o):
    out, _ = run(np.asarray(x), np.asarray(w_q), np.asarray(w_k),
                 np.asarray(w_v), np.asarray(w_o))
    return out
